# revision 49
# baseline (speedup 1.0000x reference)
"""Trainium2 Bass kernel for nn_BasicBlock (binarized CNN block).

Computes, data-parallel over the batch across 8 NeuronCores:
    out = hardtanh(BN1(bconv3x3(sign(x), sign(w1))) + x)
    out = hardtanh(BN2(bconv3x3(sign(out), sign(w2))) + out)
with training-mode BatchNorm whose statistics are all-reduced across
cores (exact global batch statistics, matching the reference).

Per-core strategy (8 images of the 64-image batch):
  - sign(x) binarized HOST-side into the zero-padded 30x30 fp8 cell
    layout, so conv1 starts immediately after a small DMA.
  - conv = 9 taps x 2 channel-group DoubleRow fp8 matmuls into PSUM
    (exact for +-1 / {0,1} inputs); conv outputs stored int16 (exact).
  - layer-2 conv input is binarized to {0,1} with a single DVE is_ge
    compare against a per-channel threshold (tau = mu - beta/s); pads
    hold 0.5 and the PSUM->SBUF copy scales by 2, which turns the
    {0,1} conv into sign-conv + a per-channel constant shift that
    training-mode BN absorbs exactly.
  - BN statistics: per-chunk bn_stats on DVE (+ ACT Square/accum for
    the earliest layer-2 chunks to balance engines), combined globally
    with a 2KB AllGather; affine (s, t, tau) derived on device.
  - elementwise dataflow is spread across DVE/ACT/Pool; residual `w`
    kept f32 so the layer-2 binarization is sign-exact; o1/out are fp16
    (host upcasts the fp16 output back to f32).
"""

import sys

if "/opt/trn_rl_repo" not in sys.path:
    sys.path.insert(0, "/opt/trn_rl_repo")

from contextlib import ExitStack

import numpy as np

import concourse.bass as bass
import concourse.mybir as mybir
from concourse.bass_utils import run_bass_kernel_spmd
from concourse.tile import TileContext

NCORES = 8
N_GLOBAL, C, H, W = 64, 256, 28, 28
NLOC = N_GLOBAL // NCORES  # 8 images per core
HP, WP = H + 2, W + 2      # zero-padded image
IMG, IMGP = H * W, HP * WP
NPIX = NLOC * IMG          # 6272 output pixels per core
IMGC = 976                 # per-image padded cell: 32 margin + 900 + 44
IOFF = 32                  # image data offset inside the cell
P = 128
KG = MG = C // P           # 2 channel groups per side
TAPS = 9
EPS = 1e-5

# asymmetric conv chunks: top = padded rows 1-15 (15 interior rows),
# bottom = rows 16-28 (13 rows); streams include the 2 pad columns.
CHA, CHB = 15 * W, 13 * W            # 420 / 364 interior px
PCHA, PCHB = 15 * WP, 13 * WP        # 450 / 390 streamed positions
NCHUNK = NLOC * 2                    # 16 chunks per layer

# layer-2 stats engine split: the FIRST ACT_CMG chunk-mg units (which
# complete earliest) use ACT Square+accum (Sy, Syy slots); the rest --
# including the late, collective-critical chunks -- use DVE bn_stats.
ACT_CMG = 8
# binarize layer-2 input on Pool (tensor_tensor is_ge vs broadcast tau)
POOL_BIN = False

F32 = mybir.dt.float32
I16 = mybir.dt.int16
FP16 = mybir.dt.float16
FP8 = mybir.dt.float8e4
AF = mybir.ActivationFunctionType
OP = mybir.AluOpType

# walrus in this container accepts at most ONE sem-wait per instruction;
# hoist extra waits onto same-engine NOPs placed just before.
MAX_WAITS = 1
_split_ctr = [0]


def legalize_waits(nc):
    for fn in nc.m.functions:
        for bb in fn.blocks:
            out = []
            for ins in list(bb.instructions):
                si = ins.sync_info
                if si is not None and len(si.on_wait) > MAX_WAITS:
                    waits = list(si.on_wait)
                    extra, keep = waits[:-MAX_WAITS], waits[-MAX_WAITS:]
                    for w in extra:
                        _split_ctr[0] += 1
                        nop = mybir.InstNoOp(
                            name=f"I-waitsplit-{_split_ctr[0]}", engine=ins.engine
                        )
                        nop.sync_info = mybir.SyncInfo(on_wait=[w], on_update=[])
                        out.append(nop)
                    ins.sync_info = mybir.SyncInfo(
                        on_wait=keep, on_update=list(si.on_update)
                    )
                out.append(ins)
            bb.instructions = out


def build(reps=1):
    nc = bass.Bass()

    xs1_ext = nc.dram_tensor("xs1", [NLOC, P, KG, IMGC], FP8, kind="ExternalInput")
    x_ext = nc.dram_tensor("x", [NLOC, C, H, W], F32, kind="ExternalInput")
    w_ext = {
        l: nc.dram_tensor(f"w{l}b", [KG, P, TAPS, MG * P], FP8, kind="ExternalInput")
        for l in (1, 2)
    }
    gm_ext = {
        l: nc.dram_tensor(f"gamma{l}", [C], F32, kind="ExternalInput") for l in (1, 2)
    }
    bt_ext = {
        l: nc.dram_tensor(f"beta{l}", [C], F32, kind="ExternalInput") for l in (1, 2)
    }
    out_ext = nc.dram_tensor("out", [NLOC, C, H, W], FP16, kind="ExternalOutput")
    cc_in = {l: nc.dram_tensor(f"cc{l}_in", [MG, 2, P], F32) for l in (1, 2)}
    cc_out = {
        l: nc.dram_tensor(f"cc{l}_out", [NCORES, MG, 2, P], F32, addr_space="Shared")
        for l in (1, 2)
    }

    xv = x_ext.rearrange("n c h w -> c n (h w)")    # [256, 8, 784]
    ov = out_ext.rearrange("n c h w -> c n h w")    # [256, 8, 28, 28] fp16

    with TileContext(nc) as tc:
        ctx = ExitStack()
        singles = ctx.enter_context(tc.tile_pool(name="singles", bufs=1))
        aring = ctx.enter_context(tc.tile_pool(name="aring", bufs=2))
        pring = ctx.enter_context(tc.tile_pool(name="pring", bufs=6))
        qring = ctx.enter_context(tc.tile_pool(name="qring", bufs=6))
        outst = ctx.enter_context(tc.tile_pool(name="outst", bufs=6))
        sqscr = ctx.enter_context(tc.tile_pool(name="sqscr", bufs=2))
        small = ctx.enter_context(tc.tile_pool(name="small", bufs=2))
        psum = ctx.enter_context(tc.tile_pool(name="psum", bufs=8, space="PSUM"))

        # ---- persistent tiles -------------------------------------------
        xs = {l: [singles.tile([P, KG, IMGC], FP8, tag=f"xs{l}n{n}", name=f"xs{l}n{n}")
                  for n in range(NLOC)] for l in (1, 2)}
        y = {l: singles.tile([P, MG, NPIX], I16, tag=f"y{l}", name=f"y{l}") for l in (1, 2)}
        wres = singles.tile([P, MG, NPIX], F32, tag="wres", name="wres")
        o1 = singles.tile([P, MG, NPIX], FP16, tag="o1", name="o1")
        wsb = {l: singles.tile([P, TAPS, KG, MG * P], FP8, tag=f"wsb{l}", name=f"wsb{l}") for l in (1, 2)}
        st = {l: singles.tile([P, MG, NCHUNK, 6], F32, tag=f"st{l}", name=f"st{l}") for l in (1, 2)}
        sySlot = singles.tile([P, MG, NCHUNK, 2], F32, tag="sySlot", name="sySlot")
        gmb = {l: singles.tile([P, MG], F32, tag=f"gmb{l}", name=f"gmb{l}") for l in (1, 2)}
        btb = {l: singles.tile([P, MG], F32, tag=f"btb{l}", name=f"btb{l}") for l in (1, 2)}
        epsb = singles.tile([P, 1], F32)
        taub = (singles.tile([P, MG, IMG], F32, tag="taub", name="taub")
                if POOL_BIN else None)

        nc.vector.memset(epsb, EPS)

        # xs2 cells: pads/margins hold 0.5 ({0,1} encoding of sign 0)
        for n in range(NLOC):
            t_ = xs[2][n]
            eng = nc.vector if n % 2 == 0 else nc.gpsimd
            eng.memset(t_[:, :, 0:IOFF + WP], 0.5)          # margin + pad row 0
            eng.memset(t_[:, :, IMGC - 44 - WP:IMGC], 0.5)  # pad row 29 + margin
            for kg in range(KG):
                border = bass.AP(
                    tensor=t_.tensor, offset=t_.offset + kg * IMGC + IOFF + WP,
                    ap=[list(t_.ap[0]), [WP, H], [WP - 1, 2]],
                )
                eng.memset(border, 0.5)

        # ---- weights / host-signed x in (conv1-critical DMAs first) -----
        nc.sync.dma_start(out=xs[1][0], in_=xs1_ext[0])
        for t in range(TAPS):
            for kg in range(KG):
                nc.sync.dma_start(out=wsb[1][:, t, kg, :], in_=w_ext[1][kg][:, t, :])
        for n in range(1, NLOC):
            nc.sync.dma_start(out=xs[1][n], in_=xs1_ext[n])
        for kg in range(KG):
            nc.sync.dma_start(out=wsb[2][:, :, kg, :], in_=w_ext[2][kg])
        for l in (1, 2):
            nc.sync.dma_start(out=gmb[l], in_=gm_ext[l].rearrange("(g p) -> p g", p=P))
            nc.sync.dma_start(out=btb[l], in_=bt_ext[l].rearrange("(g p) -> p g", p=P))
        # x f32 prefetched into wres during phase 1 (overwritten in place
        # by w = s1*y1 + x in phase 2)
        for mg in range(MG):
            nc.sync.dma_start(
                out=wres[:, mg, :].rearrange("p (n q) -> p n q", n=NLOC),
                in_=xv[mg * P:(mg + 1) * P, :, :])

        for _rep in range(reps):
            run_iteration(nc, tc, locals())
        ctx.close()

    legalize_waits(nc)
    return nc


def run_iteration(nc, tc, env):
    g = type("G", (), env)  # attribute access to captured tiles/views

    xs, y, wres, o1, wsb, st = g.xs, g.y, g.wres, g.o1, g.wsb, g.st
    sySlot, gmb, btb, epsb = g.sySlot, g.gmb, g.btb, g.epsb
    xv, ov, cc_in, cc_out = g.xv, g.ov, g.cc_in, g.cc_out
    psum, small = g.psum, g.small
    aring, pring, qring, outst, sqscr = (
        g.aring, g.pring, g.qring, g.outst, g.sqscr)

    # ---- conv chunk: 9 taps x 2 kg-pair DoubleRow matmuls per mg --------
    def conv_chunk(l, n, hb):
        pch = PCHA if hb == 0 else PCHB
        ps = {mg: psum.tile([P, PCHA], F32, tag="ps", name="ps") for mg in range(MG)}
        for t in range(TAPS):
            dy, dx = t // 3 - 1, t % 3 - 1
            q0 = IOFF + WP * (1 + 15 * hb) + WP * dy + dx
            rhs = xs[l][n][:, :, q0:q0 + pch]
            for mg in range(MG):
                lhsT = wsb[l][:, t, :, mg * P:(mg + 1) * P]
                nc.tensor.matmul(
                    ps[mg][:, :pch], lhsT, rhs,
                    start=(t == 0), stop=(t == TAPS - 1),
                    perf_mode=mybir.MatmulPerfMode.DoubleRow,
                )
        return ps

    # copy PSUM->SBUF int16 (layer2: scale=2 -> integer {0,1}-conv fix)
    # + per-chunk stats (DVE bn_stats or ACT Square/accum + copy-accum).
    def chunk_post(l, n, hb, ps):
        ci = 2 * n + hb
        rows = 15 if hb == 0 else 13
        yoff = n * IMG + (CHA if hb == 1 else 0)
        npx = CHA if hb == 0 else CHB
        for mg in range(MG):
            psv = ps[mg][:, : (PCHA if hb == 0 else PCHB)].rearrange(
                "p (r c) -> p r c", c=WP)
            interior = psv[:, :, 1:1 + W]
            ysl = y[l][:, mg, yoff:yoff + npx]
            on_dve = l == 1 or (2 * ci + mg) >= ACT_CMG
            if on_dve:
                nc.scalar.activation(
                    out=ysl.rearrange("p (r c) -> p r c", c=W),
                    in_=interior, func=AF.Copy,
                    scale=1.0 if l == 1 else 2.0,
                )
                nc.vector.bn_stats(out=st[l][:, mg, ci, :], in_=ysl)
            else:
                nc.scalar.activation(
                    out=ysl.rearrange("p (r c) -> p r c", c=W),
                    in_=interior, func=AF.Copy, scale=2.0,
                    accum_out=sySlot[:, mg, ci, 0:1],
                )
                sq = sqscr.tile([P, CHA], F32, tag="sq")
                nc.scalar.activation(
                    out=sq[:, :npx], in_=ysl, func=AF.Square,
                    accum_out=sySlot[:, mg, ci, 1:2],
                )

    # ---- global stats -> affine params ------------------------------
    def stats_and_affine(l):
        # ccsb: per-core contribution (mean, E[y^2]) / NCORES
        ccsb = small.tile([P, MG, 2], F32, tag="ccsb", name="ccsb")
        if l == 1:
            mv = small.tile([P, MG, 2], F32, tag="mv", name="mv")
            for mg in range(MG):
                nc.vector.bn_aggr(out=mv[:, mg, :], in_=st[l][:, mg, :, :])
            msq = small.tile([P, MG, 1], F32, tag="msq", name="msq")
            nc.vector.tensor_tensor(out=msq, in0=mv[:, :, 0:1], in1=mv[:, :, 0:1], op=OP.mult)
            nc.vector.tensor_tensor(out=msq, in0=mv[:, :, 1:2], in1=msq, op=OP.add)
            nc.vector.tensor_scalar(out=ccsb[:, :, 0:1], in0=mv[:, :, 0:1],
                                    scalar1=1.0 / NCORES, scalar2=None, op0=OP.mult)
            nc.vector.tensor_scalar(out=ccsb[:, :, 1:2], in0=msq,
                                    scalar1=1.0 / NCORES, scalar2=None, op0=OP.mult)
        else:
            # group A: chunk-mg < DVE_CMG via bn_aggr; group B: slot sums
            mvA = small.tile([P, MG, 2], F32, tag="mvA", name="mvA")
            cntA = []
            for mg in range(MG):
                cis = [ci for ci in range(NCHUNK) if (2 * ci + mg) >= ACT_CMG]
                cnt = sum(CHA if ci % 2 == 0 else CHB for ci in cis)
                cntA.append(cnt)
                stv = st[l][:, mg, cis[0]:cis[-1] + 1, :]
                nc.vector.bn_aggr(out=mvA[:, mg, :], in_=stv)
            syB = small.tile([P, MG, 2], F32, tag="syB", name="syB")
            for mg in range(MG):
                cis = [ci for ci in range(NCHUNK) if (2 * ci + mg) < ACT_CMG]
                slv = sySlot[:, mg, cis[0]:cis[-1] + 1, :]
                nc.vector.reduce_sum(out=syB[:, mg, :], in_=slv.rearrange("p c d -> p d c"),
                                     axis=mybir.AxisListType.X)
            # totals: S = meanA*cntA + SyB ; Q = (varA+meanA^2)*cntA + SyyB
            mA2 = small.tile([P, MG, 1], F32, tag="mA2", name="mA2")
            nc.vector.tensor_tensor(out=mA2, in0=mvA[:, :, 0:1], in1=mvA[:, :, 0:1], op=OP.mult)
            e2A = small.tile([P, MG, 1], F32, tag="e2A", name="e2A")
            nc.vector.tensor_tensor(out=e2A, in0=mvA[:, :, 1:2], in1=mA2, op=OP.add)
            for mg in range(MG):
                cA = float(cntA[mg])
                sc = 1.0 / (IMG * NLOC * NCORES)
                # ccsb0 = (meanA*cA + SyB) * sc ; ccsb1 = (e2A*cA + SyyB) * sc
                nc.vector.scalar_tensor_tensor(
                    out=ccsb[:, mg, 0:1], in0=mvA[:, mg, 0:1], scalar=cA,
                    in1=syB[:, mg, 0:1], op0=OP.mult, op1=OP.add)
                nc.vector.scalar_tensor_tensor(
                    out=ccsb[:, mg, 1:2], in0=e2A[:, mg, 0:1], scalar=cA,
                    in1=syB[:, mg, 1:2], op0=OP.mult, op1=OP.add)
            nc.vector.tensor_scalar(out=ccsb[:, :, 0:2], in0=ccsb[:, :, 0:2],
                                    scalar1=1.0 / (NPIX * NCORES), scalar2=None, op0=OP.mult)

        nc.sync.dma_start(out=cc_in[l].rearrange("g d p -> p g d"), in_=ccsb)
        nc.gpsimd.collective_compute(
            "AllGather", OP.bypass,
            ins=[cc_in[l][:, :, :]], outs=[cc_out[l][:, :, :, :]],
            replica_groups=[list(range(NCORES))],
        )
        return ccsb

    def affine(l, post_ops=None):
        glr = small.tile([P, NCORES, MG * 2], F32, tag="glr", name="glr")
        nc.sync.dma_start(out=glr,
                          in_=cc_out[l].rearrange("r g d p -> p r (g d)"))
        gl = small.tile([P, MG, 2], F32, tag="gl", name="gl")
        nc.vector.reduce_sum(out=gl, in_=glr.rearrange("p r q -> p q r"),
                             axis=mybir.AxisListType.X)
        mean, e2 = gl[:, :, 0:1], gl[:, :, 1:2]
        nvar = small.tile([P, MG, 1], F32, tag="nvar", name="nvar")
        for mg in range(MG):
            nc.vector.scalar_tensor_tensor(
                out=nvar[:, mg, :], in0=mean[:, mg, :], scalar=mean[:, mg, :],
                in1=e2[:, mg, :], op0=OP.mult, op1=OP.subtract)
        sd = small.tile([P, MG, 1], F32, tag="sd", name="sd")
        for mg in range(MG):
            nc.scalar.activation(out=sd[:, mg, :], in_=nvar[:, mg, :], func=AF.Sqrt,
                                 bias=epsb, scale=-1.0)
        sT = small.tile([P, MG, 1], F32, tag=f"sT{l}", name=f"sT{l}")
        tT = small.tile([P, MG, 1], F32, tag=f"tT{l}", name=f"tT{l}")
        rinv = small.tile([P, MG, 1], F32, tag="rinv", name="rinv")
        nc.vector.reciprocal(out=rinv, in_=sd)
        nc.vector.tensor_tensor(out=sT, in0=rinv,
                                in1=gmb[l].rearrange("p (g o) -> p g o", o=1), op=OP.mult)
        # tau = s*mu - beta (one fused op); t = -tau
        tau = small.tile([P, MG, 1], F32, tag=f"tau{l}", name=f"tau{l}")
        for mg in range(MG):
            nc.vector.scalar_tensor_tensor(
                out=tau[:, mg, :], in0=mean[:, mg, :], scalar=sT[:, mg, :],
                in1=btb[l].rearrange("p (g o) -> p g o", o=1)[:, mg, :],
                op0=OP.mult, op1=OP.subtract)
        nc.vector.tensor_scalar(out=tT, in0=tau, scalar1=-1.0, scalar2=None,
                                op0=OP.mult)
        extra = {} if post_ops != "l1" else {"tau": tau}
        return {"s": sT, "t": tT, **extra}

    # ================== pipeline ==================
    # phase 1: conv1 + copies + stats
    for n in range(NLOC):
        for hb in (0, 1):
            ps = conv_chunk(1, n, hb)
            chunk_post(1, n, hb, ps)

    ccsb1 = stats_and_affine(1)
    aff1 = affine(1, post_ops="l1")
    s1, t1, tau1 = aff1["s"], aff1["t"], aff1["tau"]

    yv = {l: y[l].rearrange("p m (n q) -> p m n q", n=NLOC) for l in (1, 2)}
    wv = wres.rearrange("p m (n q) -> p m n q", n=NLOC)
    o1v = o1.rearrange("p m (n q) -> p m n q", n=NLOC)

    # phase 2: per image: x in, w = x + s1*y1 (f32), binarize {0,1};
    # conv2 chunks trail 2 images behind (software pipeline).
    def b1_image(n):
        xs2v = xs[2][n][:, :, IOFF:IOFF + IMGP].rearrange("p g (r c) -> p g r c", r=HP)
        splits = [(0, 16), (16, H)] if n == 0 else [(0, H)]
        for mg in range(MG):
            wsl = wv[:, mg, n, :]
            for r0, r1 in splits:
                nc.vector.scalar_tensor_tensor(
                    out=wsl[:, r0 * W:r1 * W], in0=yv[1][:, mg, n, r0 * W:r1 * W],
                    scalar=s1[:, mg, :],
                    in1=wsl[:, r0 * W:r1 * W], op0=OP.mult, op1=OP.add)
            for r0, r1 in splits:
                if POOL_BIN and n > 0:
                    nc.gpsimd.tensor_tensor(
                        out=xs2v[:, mg, 1 + r0:1 + r1, 1:1 + W],
                        in0=wsl.rearrange("p (r c) -> p r c", c=W)[:, r0:r1, :],
                        in1=g.taub[:, mg, :].rearrange("p (r c) -> p r c", c=W)[:, r0:r1, :],
                        op=OP.is_ge)
                else:
                    nc.vector.tensor_scalar(
                        out=xs2v[:, mg, 1 + r0:1 + r1, 1:1 + W],
                        in0=wsl.rearrange("p (r c) -> p r c", c=W)[:, r0:r1, :],
                        scalar1=tau1[:, mg, :], scalar2=None, op0=OP.is_ge)

    def conv2_image(n):
        for hb in (0, 1):
            ps = conv_chunk(2, n, hb)
            chunk_post(2, n, hb, ps)

    for n in range(NLOC):
        b1_image(n)
        if POOL_BIN and n == 0:
            for mg in range(MG):
                nc.vector.tensor_scalar(out=g.taub[:, mg, :],
                                        in0=y[1][:, mg, 0:IMG],
                                        scalar1=0.0, scalar2=tau1[:, mg, :],
                                        op0=OP.mult, op1=OP.add)
        if n >= 2:
            conv2_image(n - 2)
    for n in range(NLOC - 2, NLOC):
        conv2_image(n)

    # gap 2: exchange stats; meanwhile o1 = clip(w + t1) (ACT/DVE split)
    ccsb2 = stats_and_affine(2)
    for n in range(NLOC):
        for mg in range(MG):
            a = aring.tile([P, IMG], FP16, tag="a")
            nc.scalar.activation(out=a, in_=wv[:, mg, n, :], func=AF.Identity,
                                 bias=t1[:, mg, :], scale=1.0)
            nc.vector.tensor_scalar(out=o1v[:, mg, n, :], in0=a,
                                    scalar1=1.0, scalar2=-1.0, op0=OP.min, op1=OP.max)
    aff2 = affine(2)
    s2, t2 = aff2["s"], aff2["t"]

    # tail: out = clip(s2*y2 + t2 + o1) -> fp16 -> DRAM
    unit = 0
    for n in range(NLOC):
        for mg in range(MG):
            p_ = pring.tile([P, IMG], FP16, tag="p")
            if unit < 14:
                nc.scalar.activation(out=p_, in_=yv[2][:, mg, n, :], func=AF.Identity,
                                     bias=t2[:, mg, :], scale=s2[:, mg, :])
            else:
                nc.vector.tensor_scalar(out=p_, in0=yv[2][:, mg, n, :],
                                        scalar1=s2[:, mg, :], scalar2=t2[:, mg, :],
                                        op0=OP.mult, op1=OP.add)
            unit += 1
            q_ = qring.tile([P, IMG], FP16, tag="q")
            nc.vector.tensor_tensor(out=q_, in0=p_, in1=o1v[:, mg, n, :], op=OP.add)
            oc = outst.tile([P, IMG], FP16, tag="oc")
            nc.vector.tensor_scalar(out=oc, in0=q_, scalar1=1.0, scalar2=-1.0,
                                    op0=OP.min, op1=OP.max)
            nc.sync.dma_start(
                out=ov[mg * P:(mg + 1) * P, n, :, :],
                in_=oc.rearrange("p (r c) -> p r c", c=W),
            )


_CACHE = {}


def prep_inputs(x, w1, gamma1, beta1, w2, gamma2, beta2):
    fp8np = mybir.dt.np(FP8)

    def prep_w(w):
        wb = np.where(np.asarray(w) >= 0, 1.0, -1.0).astype(np.float32)
        t = wb.reshape(MG, P, KG, P, 3, 3)       # [mg, m, kg, k, ky, kx]
        arr = t.transpose(2, 3, 4, 5, 0, 1)      # [kg, k, ky, kx, mg, m]
        return np.ascontiguousarray(arr.reshape(KG, P, TAPS, MG * P)).astype(fp8np)

    x = np.asarray(x, dtype=np.float32)
    w1b, w2b = prep_w(w1), prep_w(w2)
    g1 = np.asarray(gamma1, np.float32); b1 = np.asarray(beta1, np.float32)
    g2 = np.asarray(gamma2, np.float32); b2 = np.asarray(beta2, np.float32)

    # host-side sign(x) packed into the padded per-image fp8 cell layout
    xs_sign = np.where(x >= 0, 1.0, -1.0).astype(np.float32)
    in_maps = []
    for c in range(NCORES):
        xl = x[c * NLOC:(c + 1) * NLOC]
        sl = xs_sign[c * NLOC:(c + 1) * NLOC]       # [NLOC, C, H, W]
        cell = np.zeros((NLOC, P, KG, IMGC), np.float32)
        s4 = sl.reshape(NLOC, KG, P, H, W)
        pad = np.zeros((NLOC, KG, P, HP, WP), np.float32)
        pad[:, :, :, 1:1 + H, 1:1 + W] = s4
        cell[:, :, :, IOFF:IOFF + IMGP] = (
            pad.transpose(0, 2, 1, 3, 4).reshape(NLOC, P, KG, IMGP))
        in_maps.append({
            "xs1": cell.astype(fp8np),
            "x": np.ascontiguousarray(xl),
            "w1b": w1b, "w2b": w2b,
            "gamma1": g1, "beta1": b1, "gamma2": g2, "beta2": b2,
        })
    return in_maps


def kernel(x, w1, gamma1, beta1, w2, gamma2, beta2):
    if "nc" not in _CACHE:
        _CACHE["nc"] = build()
    nc = _CACHE["nc"]
    in_maps = prep_inputs(x, w1, gamma1, beta1, w2, gamma2, beta2)
    res = run_bass_kernel_spmd(nc, in_maps, core_ids=list(range(NCORES)))
    return np.concatenate(
        [res.results[c]["out"] for c in range(NCORES)], axis=0
    ).astype(np.float32)


# revision 51
# speedup vs baseline: 1.2378x; 1.2378x over previous
"""Trainium2 Bass kernel for nn_BasicBlock (binarized CNN block).

Computes, data-parallel over the batch across 8 NeuronCores:
    out = hardtanh(BN1(bconv3x3(sign(x), sign(w1))) + x)
    out = hardtanh(BN2(bconv3x3(sign(out), sign(w2))) + out)
with training-mode BatchNorm whose statistics are all-reduced across
cores (exact global batch statistics, matching the reference).

Per-core strategy (8 images of the 64-image batch):
  - sign(x) binarized HOST-side into the zero-padded 30x30 fp8 cell
    layout, so conv1 starts immediately after a small DMA.
  - conv = 9 taps x 2 channel-group DoubleRow fp8 matmuls into PSUM
    (exact for +-1 / {0,1} inputs); conv outputs stored int16 (exact).
  - layer-2 conv input is binarized to {0,1} with a single DVE is_ge
    compare against a per-channel threshold (tau = mu - beta/s); pads
    hold 0.5 and the PSUM->SBUF copy scales by 2, which turns the
    {0,1} conv into sign-conv + a per-channel constant shift that
    training-mode BN absorbs exactly.
  - BN statistics: per-chunk bn_stats on DVE (+ ACT Square/accum for
    the earliest layer-2 chunks to balance engines), combined globally
    with a 2KB AllGather; affine (s, t, tau) derived on device.
  - elementwise dataflow is spread across DVE/ACT/Pool; residual `w`
    kept f32 so the layer-2 binarization is sign-exact; o1/out are fp16
    (host upcasts the fp16 output back to f32).
"""

import sys

if "/opt/trn_rl_repo" not in sys.path:
    sys.path.insert(0, "/opt/trn_rl_repo")

from contextlib import ExitStack

import numpy as np

import concourse.bass as bass
import concourse.mybir as mybir
from concourse.bass_utils import run_bass_kernel_spmd
from concourse.tile import TileContext

NCORES = 8
N_GLOBAL, C, H, W = 64, 256, 28, 28
NLOC = N_GLOBAL // NCORES  # 8 images per core
HP, WP = H + 2, W + 2      # zero-padded image
IMG, IMGP = H * W, HP * WP
NPIX = NLOC * IMG          # 6272 output pixels per core
IMGC = 976                 # per-image padded cell: 32 margin + 900 + 44
IOFF = 32                  # image data offset inside the cell
P = 128
KG = MG = C // P           # 2 channel groups per side
TAPS = 9
EPS = 1e-5

# asymmetric conv chunks: top = padded rows 1-15 (15 interior rows),
# bottom = rows 16-28 (13 rows); streams include the 2 pad columns.
CHA, CHB = 15 * W, 13 * W            # 420 / 364 interior px
PCHA, PCHB = 15 * WP, 13 * WP        # 450 / 390 streamed positions
NCHUNK = NLOC * 2                    # 16 chunks per layer

# layer-2 stats engine split: the FIRST ACT_CMG chunk-mg units (which
# complete earliest) use ACT Square+accum (Sy, Syy slots); the rest --
# including the late, collective-critical chunks -- use DVE bn_stats.
ACT_CMG = 8
# binarize layer-2 input on Pool (tensor_tensor is_ge vs broadcast tau)
POOL_BIN = False

F32 = mybir.dt.float32
I16 = mybir.dt.int16
FP16 = mybir.dt.float16
FP8 = mybir.dt.float8e4
AF = mybir.ActivationFunctionType
OP = mybir.AluOpType

# walrus in this container accepts at most ONE sem-wait per instruction;
# hoist extra waits onto same-engine NOPs placed just before.
MAX_WAITS = 1
_split_ctr = [0]


def legalize_waits(nc):
    for fn in nc.m.functions:
        for bb in fn.blocks:
            out = []
            for ins in list(bb.instructions):
                si = ins.sync_info
                if si is not None and len(si.on_wait) > MAX_WAITS:
                    waits = list(si.on_wait)
                    extra, keep = waits[:-MAX_WAITS], waits[-MAX_WAITS:]
                    for w in extra:
                        _split_ctr[0] += 1
                        nop = mybir.InstNoOp(
                            name=f"I-waitsplit-{_split_ctr[0]}", engine=ins.engine
                        )
                        nop.sync_info = mybir.SyncInfo(on_wait=[w], on_update=[])
                        out.append(nop)
                    ins.sync_info = mybir.SyncInfo(
                        on_wait=keep, on_update=list(si.on_update)
                    )
                out.append(ins)
            bb.instructions = out


def build(reps=1):
    nc = bass.Bass()

    xs1_ext = nc.dram_tensor("xs1", [NLOC, P, KG, IMGC], FP8, kind="ExternalInput")
    x_ext = nc.dram_tensor("x", [NLOC, C, H, W], F32, kind="ExternalInput")
    w_ext = {
        l: nc.dram_tensor(f"w{l}b", [KG, P, TAPS, MG * P], FP8, kind="ExternalInput")
        for l in (1, 2)
    }
    gm_ext = {
        l: nc.dram_tensor(f"gamma{l}", [C], F32, kind="ExternalInput") for l in (1, 2)
    }
    bt_ext = {
        l: nc.dram_tensor(f"beta{l}", [C], F32, kind="ExternalInput") for l in (1, 2)
    }
    out_ext = nc.dram_tensor("out", [NLOC, C, H, W], FP16, kind="ExternalOutput")
    cc_in = {l: nc.dram_tensor(f"cc{l}_in", [MG, 2, P], F32) for l in (1, 2)}
    cc_out = {
        l: nc.dram_tensor(f"cc{l}_out", [NCORES, MG, 2, P], F32, addr_space="Shared")
        for l in (1, 2)
    }

    xv = x_ext.rearrange("n c h w -> c n (h w)")    # [256, 8, 784]
    ov = out_ext.rearrange("n c h w -> c n h w")    # [256, 8, 28, 28] fp16

    with TileContext(nc) as tc:
        ctx = ExitStack()
        singles = ctx.enter_context(tc.tile_pool(name="singles", bufs=1))
        aring = ctx.enter_context(tc.tile_pool(name="aring", bufs=2))
        pring = ctx.enter_context(tc.tile_pool(name="pring", bufs=3))
        qring = ctx.enter_context(tc.tile_pool(name="qring", bufs=3))
        outst = ctx.enter_context(tc.tile_pool(name="outst", bufs=3))
        sqscr = ctx.enter_context(tc.tile_pool(name="sqscr", bufs=2))
        small = ctx.enter_context(tc.tile_pool(name="small", bufs=2))
        psum = ctx.enter_context(tc.tile_pool(name="psum", bufs=8, space="PSUM"))

        # ---- persistent tiles -------------------------------------------
        xs = {l: [singles.tile([P, KG, IMGC], FP8, tag=f"xs{l}n{n}", name=f"xs{l}n{n}")
                  for n in range(NLOC)] for l in (1, 2)}
        y = {l: singles.tile([P, MG, NPIX], I16, tag=f"y{l}", name=f"y{l}") for l in (1, 2)}
        wres = singles.tile([P, MG, NPIX], F32, tag="wres", name="wres")
        o1 = singles.tile([P, MG, NPIX], FP16, tag="o1", name="o1")
        wsb = {l: singles.tile([P, TAPS, KG, MG * P], FP8, tag=f"wsb{l}", name=f"wsb{l}") for l in (1, 2)}
        st = {l: singles.tile([P, MG, NCHUNK, 6], F32, tag=f"st{l}", name=f"st{l}") for l in (1, 2)}
        sySlot = singles.tile([P, MG, NCHUNK, 2], F32, tag="sySlot", name="sySlot")
        gmb = {l: singles.tile([P, MG], F32, tag=f"gmb{l}", name=f"gmb{l}") for l in (1, 2)}
        btb = {l: singles.tile([P, MG], F32, tag=f"btb{l}", name=f"btb{l}") for l in (1, 2)}
        epsb = singles.tile([P, 1], F32)
        taub = (singles.tile([P, MG, IMG], F32, tag="taub", name="taub")
                if POOL_BIN else None)

        nc.vector.memset(epsb, EPS)

        # xs2 cells: pads/margins hold 0.5 ({0,1} encoding of sign 0)
        for n in range(NLOC):
            t_ = xs[2][n]
            eng = nc.vector if n % 2 == 0 else nc.gpsimd
            eng.memset(t_[:, :, 0:IOFF + WP], 0.5)          # margin + pad row 0
            eng.memset(t_[:, :, IMGC - 44 - WP:IMGC], 0.5)  # pad row 29 + margin
            for kg in range(KG):
                border = bass.AP(
                    tensor=t_.tensor, offset=t_.offset + kg * IMGC + IOFF + WP,
                    ap=[list(t_.ap[0]), [WP, H], [WP - 1, 2]],
                )
                eng.memset(border, 0.5)

        # ---- weights / host-signed x in (conv1-critical DMAs first) -----
        nc.sync.dma_start(out=xs[1][0], in_=xs1_ext[0])
        for t in range(TAPS):
            for kg in range(KG):
                nc.sync.dma_start(out=wsb[1][:, t, kg, :], in_=w_ext[1][kg][:, t, :])
        for n in range(1, NLOC):
            nc.sync.dma_start(out=xs[1][n], in_=xs1_ext[n])
        for kg in range(KG):
            nc.sync.dma_start(out=wsb[2][:, :, kg, :], in_=w_ext[2][kg])
        for l in (1, 2):
            nc.sync.dma_start(out=gmb[l], in_=gm_ext[l].rearrange("(g p) -> p g", p=P))
            nc.sync.dma_start(out=btb[l], in_=bt_ext[l].rearrange("(g p) -> p g", p=P))
        # x f32 prefetched into wres during phase 1 (overwritten in place
        # by w = s1*y1 + x in phase 2)
        for mg in range(MG):
            nc.sync.dma_start(
                out=wres[:, mg, :].rearrange("p (n q) -> p n q", n=NLOC),
                in_=xv[mg * P:(mg + 1) * P, :, :])

        for _rep in range(reps):
            run_iteration(nc, tc, locals())
        ctx.close()

    legalize_waits(nc)
    return nc


def run_iteration(nc, tc, env):
    g = type("G", (), env)  # attribute access to captured tiles/views

    xs, y, wres, o1, wsb, st = g.xs, g.y, g.wres, g.o1, g.wsb, g.st
    sySlot, gmb, btb, epsb = g.sySlot, g.gmb, g.btb, g.epsb
    xv, ov, cc_in, cc_out = g.xv, g.ov, g.cc_in, g.cc_out
    psum, small = g.psum, g.small
    aring, pring, qring, outst, sqscr = (
        g.aring, g.pring, g.qring, g.outst, g.sqscr)

    # ---- conv chunk: 9 taps x 2 kg-pair DoubleRow matmuls per mg --------
    def conv_chunk(l, n, hb):
        pch = PCHA if hb == 0 else PCHB
        ps = {mg: psum.tile([P, PCHA], F32, tag="ps", name="ps") for mg in range(MG)}
        for t in range(TAPS):
            dy, dx = t // 3 - 1, t % 3 - 1
            q0 = IOFF + WP * (1 + 15 * hb) + WP * dy + dx
            rhs = xs[l][n][:, :, q0:q0 + pch]
            for mg in range(MG):
                lhsT = wsb[l][:, t, :, mg * P:(mg + 1) * P]
                nc.tensor.matmul(
                    ps[mg][:, :pch], lhsT, rhs,
                    start=(t == 0), stop=(t == TAPS - 1),
                    perf_mode=mybir.MatmulPerfMode.DoubleRow,
                )
        return ps

    # copy PSUM->SBUF int16 (layer2: scale=2 -> integer {0,1}-conv fix)
    # + per-chunk stats (DVE bn_stats or ACT Square/accum + copy-accum).
    def chunk_post(l, n, hb, ps):
        ci = 2 * n + hb
        rows = 15 if hb == 0 else 13
        yoff = n * IMG + (CHA if hb == 1 else 0)
        npx = CHA if hb == 0 else CHB
        for mg in range(MG):
            psv = ps[mg][:, : (PCHA if hb == 0 else PCHB)].rearrange(
                "p (r c) -> p r c", c=WP)
            interior = psv[:, :, 1:1 + W]
            ysl = y[l][:, mg, yoff:yoff + npx]
            on_dve = l == 1 or (2 * ci + mg) >= ACT_CMG
            if on_dve:
                nc.scalar.activation(
                    out=ysl.rearrange("p (r c) -> p r c", c=W),
                    in_=interior, func=AF.Copy,
                    scale=1.0 if l == 1 else 2.0,
                )
                nc.vector.bn_stats(out=st[l][:, mg, ci, :], in_=ysl)
            else:
                nc.scalar.activation(
                    out=ysl.rearrange("p (r c) -> p r c", c=W),
                    in_=interior, func=AF.Copy, scale=2.0,
                    accum_out=sySlot[:, mg, ci, 0:1],
                )
                sq = sqscr.tile([P, CHA], F32, tag="sq")
                nc.scalar.activation(
                    out=sq[:, :npx], in_=ysl, func=AF.Square,
                    accum_out=sySlot[:, mg, ci, 1:2],
                )

    # ---- global stats -> affine params ------------------------------
    def stats_and_affine(l):
        # ccsb: per-core contribution (mean, E[y^2]) / NCORES
        ccsb = small.tile([P, MG, 2], F32, tag="ccsb", name="ccsb")
        if l == 1:
            mv = small.tile([P, MG, 2], F32, tag="mv", name="mv")
            for mg in range(MG):
                nc.vector.bn_aggr(out=mv[:, mg, :], in_=st[l][:, mg, :, :])
            msq = small.tile([P, MG, 1], F32, tag="msq", name="msq")
            nc.vector.tensor_tensor(out=msq, in0=mv[:, :, 0:1], in1=mv[:, :, 0:1], op=OP.mult)
            nc.vector.tensor_tensor(out=msq, in0=mv[:, :, 1:2], in1=msq, op=OP.add)
            nc.vector.tensor_scalar(out=ccsb[:, :, 0:1], in0=mv[:, :, 0:1],
                                    scalar1=1.0 / NCORES, scalar2=None, op0=OP.mult)
            nc.vector.tensor_scalar(out=ccsb[:, :, 1:2], in0=msq,
                                    scalar1=1.0 / NCORES, scalar2=None, op0=OP.mult)
        else:
            # group A: chunk-mg < DVE_CMG via bn_aggr; group B: slot sums
            mvA = small.tile([P, MG, 2], F32, tag="mvA", name="mvA")
            cntA = []
            for mg in range(MG):
                cis = [ci for ci in range(NCHUNK) if (2 * ci + mg) >= ACT_CMG]
                cnt = sum(CHA if ci % 2 == 0 else CHB for ci in cis)
                cntA.append(cnt)
                stv = st[l][:, mg, cis[0]:cis[-1] + 1, :]
                nc.vector.bn_aggr(out=mvA[:, mg, :], in_=stv)
            syB = small.tile([P, MG, 2], F32, tag="syB", name="syB")
            for mg in range(MG):
                cis = [ci for ci in range(NCHUNK) if (2 * ci + mg) < ACT_CMG]
                slv = sySlot[:, mg, cis[0]:cis[-1] + 1, :]
                nc.vector.reduce_sum(out=syB[:, mg, :], in_=slv.rearrange("p c d -> p d c"),
                                     axis=mybir.AxisListType.X)
            # totals: S = meanA*cntA + SyB ; Q = (varA+meanA^2)*cntA + SyyB
            mA2 = small.tile([P, MG, 1], F32, tag="mA2", name="mA2")
            nc.vector.tensor_tensor(out=mA2, in0=mvA[:, :, 0:1], in1=mvA[:, :, 0:1], op=OP.mult)
            e2A = small.tile([P, MG, 1], F32, tag="e2A", name="e2A")
            nc.vector.tensor_tensor(out=e2A, in0=mvA[:, :, 1:2], in1=mA2, op=OP.add)
            for mg in range(MG):
                cA = float(cntA[mg])
                sc = 1.0 / (IMG * NLOC * NCORES)
                # ccsb0 = (meanA*cA + SyB) * sc ; ccsb1 = (e2A*cA + SyyB) * sc
                nc.vector.scalar_tensor_tensor(
                    out=ccsb[:, mg, 0:1], in0=mvA[:, mg, 0:1], scalar=cA,
                    in1=syB[:, mg, 0:1], op0=OP.mult, op1=OP.add)
                nc.vector.scalar_tensor_tensor(
                    out=ccsb[:, mg, 1:2], in0=e2A[:, mg, 0:1], scalar=cA,
                    in1=syB[:, mg, 1:2], op0=OP.mult, op1=OP.add)
            nc.vector.tensor_scalar(out=ccsb[:, :, 0:2], in0=ccsb[:, :, 0:2],
                                    scalar1=1.0 / (NPIX * NCORES), scalar2=None, op0=OP.mult)

        nc.sync.dma_start(out=cc_in[l].rearrange("g d p -> p g d"), in_=ccsb)
        nc.gpsimd.collective_compute(
            "AllGather", OP.bypass,
            ins=[cc_in[l][:, :, :]], outs=[cc_out[l][:, :, :, :]],
            replica_groups=[list(range(NCORES))],
        )
        return ccsb

    def affine(l, post_ops=None):
        glr = small.tile([P, NCORES, MG * 2], F32, tag="glr", name="glr")
        nc.sync.dma_start(out=glr,
                          in_=cc_out[l].rearrange("r g d p -> p r (g d)"))
        gl = small.tile([P, MG, 2], F32, tag="gl", name="gl")
        nc.vector.reduce_sum(out=gl, in_=glr.rearrange("p r q -> p q r"),
                             axis=mybir.AxisListType.X)
        mean, e2 = gl[:, :, 0:1], gl[:, :, 1:2]
        nvar = small.tile([P, MG, 1], F32, tag="nvar", name="nvar")
        for mg in range(MG):
            nc.vector.scalar_tensor_tensor(
                out=nvar[:, mg, :], in0=mean[:, mg, :], scalar=mean[:, mg, :],
                in1=e2[:, mg, :], op0=OP.mult, op1=OP.subtract)
        sd = small.tile([P, MG, 1], F32, tag="sd", name="sd")
        for mg in range(MG):
            nc.scalar.activation(out=sd[:, mg, :], in_=nvar[:, mg, :], func=AF.Sqrt,
                                 bias=epsb, scale=-1.0)
        sT = small.tile([P, MG, 1], F32, tag=f"sT{l}", name=f"sT{l}")
        tT = small.tile([P, MG, 1], F32, tag=f"tT{l}", name=f"tT{l}")
        rinv = small.tile([P, MG, 1], F32, tag="rinv", name="rinv")
        nc.vector.reciprocal(out=rinv, in_=sd)
        nc.vector.tensor_tensor(out=sT, in0=rinv,
                                in1=gmb[l].rearrange("p (g o) -> p g o", o=1), op=OP.mult)
        # tau = s*mu - beta (one fused op); t = -tau
        tau = small.tile([P, MG, 1], F32, tag=f"tau{l}", name=f"tau{l}")
        for mg in range(MG):
            nc.vector.scalar_tensor_tensor(
                out=tau[:, mg, :], in0=mean[:, mg, :], scalar=sT[:, mg, :],
                in1=btb[l].rearrange("p (g o) -> p g o", o=1)[:, mg, :],
                op0=OP.mult, op1=OP.subtract)
        nc.vector.tensor_scalar(out=tT, in0=tau, scalar1=-1.0, scalar2=None,
                                op0=OP.mult)
        extra = {} if post_ops != "l1" else {"tau": tau}
        return {"s": sT, "t": tT, **extra}

    # ================== pipeline ==================
    # phase 1: conv1 + copies + stats
    for n in range(NLOC):
        for hb in (0, 1):
            ps = conv_chunk(1, n, hb)
            chunk_post(1, n, hb, ps)

    ccsb1 = stats_and_affine(1)
    aff1 = affine(1, post_ops="l1")
    s1, t1, tau1 = aff1["s"], aff1["t"], aff1["tau"]

    yv = {l: y[l].rearrange("p m (n q) -> p m n q", n=NLOC) for l in (1, 2)}
    wv = wres.rearrange("p m (n q) -> p m n q", n=NLOC)
    o1v = o1.rearrange("p m (n q) -> p m n q", n=NLOC)

    # phase 2: per image: x in, w = x + s1*y1 (f32), binarize {0,1};
    # conv2 chunks trail 2 images behind (software pipeline).
    def b1_image(n):
        xs2v = xs[2][n][:, :, IOFF:IOFF + IMGP].rearrange("p g (r c) -> p g r c", r=HP)
        splits = [(0, 16), (16, H)] if n == 0 else [(0, H)]
        for mg in range(MG):
            wsl = wv[:, mg, n, :]
            for r0, r1 in splits:
                nc.vector.scalar_tensor_tensor(
                    out=wsl[:, r0 * W:r1 * W], in0=yv[1][:, mg, n, r0 * W:r1 * W],
                    scalar=s1[:, mg, :],
                    in1=wsl[:, r0 * W:r1 * W], op0=OP.mult, op1=OP.add)
            for r0, r1 in splits:
                if POOL_BIN and n > 0:
                    nc.gpsimd.tensor_tensor(
                        out=xs2v[:, mg, 1 + r0:1 + r1, 1:1 + W],
                        in0=wsl.rearrange("p (r c) -> p r c", c=W)[:, r0:r1, :],
                        in1=g.taub[:, mg, :].rearrange("p (r c) -> p r c", c=W)[:, r0:r1, :],
                        op=OP.is_ge)
                else:
                    nc.vector.tensor_scalar(
                        out=xs2v[:, mg, 1 + r0:1 + r1, 1:1 + W],
                        in0=wsl.rearrange("p (r c) -> p r c", c=W)[:, r0:r1, :],
                        scalar1=tau1[:, mg, :], scalar2=None, op0=OP.is_ge)

    def conv2_image(n):
        for hb in (0, 1):
            ps = conv_chunk(2, n, hb)
            chunk_post(2, n, hb, ps)

    for n in range(NLOC):
        b1_image(n)
        if POOL_BIN and n == 0:
            for mg in range(MG):
                nc.vector.tensor_scalar(out=g.taub[:, mg, :],
                                        in0=y[1][:, mg, 0:IMG],
                                        scalar1=0.0, scalar2=tau1[:, mg, :],
                                        op0=OP.mult, op1=OP.add)
        if n >= 2:
            conv2_image(n - 2)
    for n in range(NLOC - 2, NLOC):
        conv2_image(n)

    # gap 2: exchange stats; meanwhile o1 = clip(w + t1) (ACT/DVE split)
    ccsb2 = stats_and_affine(2)
    for n2 in range(NLOC // 2):
        lo, hi = 2 * n2 * IMG, (2 * n2 + 2) * IMG
        for mg in range(MG):
            a = aring.tile([P, 2 * IMG], FP16, tag="a")
            nc.scalar.activation(out=a, in_=wres[:, mg, lo:hi], func=AF.Identity,
                                 bias=t1[:, mg, :], scale=1.0)
            nc.vector.tensor_scalar(out=o1[:, mg, lo:hi], in0=a,
                                    scalar1=1.0, scalar2=-1.0, op0=OP.min, op1=OP.max)
    aff2 = affine(2)
    s2, t2 = aff2["s"], aff2["t"]

    # tail: out = clip(s2*y2 + t2 + o1) -> fp16 -> DRAM
    for n2 in range(NLOC // 2):
        lo, hi = 2 * n2 * IMG, (2 * n2 + 2) * IMG
        for mg in range(MG):
            p_ = pring.tile([P, 2 * IMG], FP16, tag="p")
            nc.scalar.activation(out=p_, in_=y[2][:, mg, lo:hi], func=AF.Identity,
                                 bias=t2[:, mg, :], scale=s2[:, mg, :])
            q_ = qring.tile([P, 2 * IMG], FP16, tag="q")
            nc.vector.tensor_tensor(out=q_, in0=p_, in1=o1[:, mg, lo:hi], op=OP.add)
            oc = outst.tile([P, 2 * IMG], FP16, tag="oc")
            nc.vector.tensor_scalar(out=oc, in0=q_, scalar1=1.0, scalar2=-1.0,
                                    op0=OP.min, op1=OP.max)
            for j in (0, 1):
                nc.sync.dma_start(
                    out=ov[mg * P:(mg + 1) * P, 2 * n2 + j, :, :],
                    in_=oc[:, j * IMG:(j + 1) * IMG].rearrange("p (r c) -> p r c", c=W),
                )


_CACHE = {}


def prep_inputs(x, w1, gamma1, beta1, w2, gamma2, beta2):
    fp8np = mybir.dt.np(FP8)

    def prep_w(w):
        wb = np.where(np.asarray(w) >= 0, 1.0, -1.0).astype(np.float32)
        t = wb.reshape(MG, P, KG, P, 3, 3)       # [mg, m, kg, k, ky, kx]
        arr = t.transpose(2, 3, 4, 5, 0, 1)      # [kg, k, ky, kx, mg, m]
        return np.ascontiguousarray(arr.reshape(KG, P, TAPS, MG * P)).astype(fp8np)

    x = np.asarray(x, dtype=np.float32)
    w1b, w2b = prep_w(w1), prep_w(w2)
    g1 = np.asarray(gamma1, np.float32); b1 = np.asarray(beta1, np.float32)
    g2 = np.asarray(gamma2, np.float32); b2 = np.asarray(beta2, np.float32)

    # host-side sign(x) packed into the padded per-image fp8 cell layout
    xs_sign = np.where(x >= 0, 1.0, -1.0).astype(np.float32)
    in_maps = []
    for c in range(NCORES):
        xl = x[c * NLOC:(c + 1) * NLOC]
        sl = xs_sign[c * NLOC:(c + 1) * NLOC]       # [NLOC, C, H, W]
        cell = np.zeros((NLOC, P, KG, IMGC), np.float32)
        s4 = sl.reshape(NLOC, KG, P, H, W)
        pad = np.zeros((NLOC, KG, P, HP, WP), np.float32)
        pad[:, :, :, 1:1 + H, 1:1 + W] = s4
        cell[:, :, :, IOFF:IOFF + IMGP] = (
            pad.transpose(0, 2, 1, 3, 4).reshape(NLOC, P, KG, IMGP))
        in_maps.append({
            "xs1": cell.astype(fp8np),
            "x": np.ascontiguousarray(xl),
            "w1b": w1b, "w2b": w2b,
            "gamma1": g1, "beta1": b1, "gamma2": g2, "beta2": b2,
        })
    return in_maps


def kernel(x, w1, gamma1, beta1, w2, gamma2, beta2):
    if "nc" not in _CACHE:
        _CACHE["nc"] = build()
    nc = _CACHE["nc"]
    in_maps = prep_inputs(x, w1, gamma1, beta1, w2, gamma2, beta2)
    res = run_bass_kernel_spmd(nc, in_maps, core_ids=list(range(NCORES)))
    return np.concatenate(
        [res.results[c]["out"] for c in range(NCORES)], axis=0
    ).astype(np.float32)


# revision 53
# speedup vs baseline: 1.3282x; 1.0731x over previous
"""Trainium2 Bass kernel for nn_BasicBlock (binarized CNN block).

Computes, data-parallel over the batch across 8 NeuronCores:
    out = hardtanh(BN1(bconv3x3(sign(x), sign(w1))) + x)
    out = hardtanh(BN2(bconv3x3(sign(out), sign(w2))) + out)
with training-mode BatchNorm whose statistics are all-reduced across
cores (exact global batch statistics, matching the reference).

Per-core strategy (8 images of the 64-image batch):
  - sign(x) binarized HOST-side into the zero-padded 30x30 fp8 cell
    layout, so conv1 starts immediately after a small DMA.
  - conv = 9 taps x 2 channel-group DoubleRow fp8 matmuls into PSUM
    (exact for +-1 / {0,1} inputs); conv outputs stored int16 (exact).
  - layer-2 conv input is binarized to {0,1} with a single DVE is_ge
    compare against a per-channel threshold (tau = mu - beta/s); pads
    hold 0.5 and the PSUM->SBUF copy scales by 2, which turns the
    {0,1} conv into sign-conv + a per-channel constant shift that
    training-mode BN absorbs exactly.
  - BN statistics: per-chunk bn_stats on DVE (+ ACT Square/accum for
    the earliest layer-2 chunks to balance engines), combined globally
    with a 2KB AllGather; affine (s, t, tau) derived on device.
  - elementwise dataflow is spread across DVE/ACT/Pool; residual `w`
    kept f32 so the layer-2 binarization is sign-exact; o1/out are fp16
    (host upcasts the fp16 output back to f32).
"""

import sys

if "/opt/trn_rl_repo" not in sys.path:
    sys.path.insert(0, "/opt/trn_rl_repo")

from contextlib import ExitStack

import numpy as np

import concourse.bass as bass
import concourse.mybir as mybir
from concourse.bass_utils import run_bass_kernel_spmd
from concourse.tile import TileContext

NCORES = 8
N_GLOBAL, C, H, W = 64, 256, 28, 28
NLOC = N_GLOBAL // NCORES  # 8 images per core
HP, WP = H + 2, W + 2      # zero-padded image
IMG, IMGP = H * W, HP * WP
NPIX = NLOC * IMG          # 6272 output pixels per core
IMGC = 976                 # per-image padded cell: 32 margin + 900 + 44
IOFF = 32                  # image data offset inside the cell
P = 128
KG = MG = C // P           # 2 channel groups per side
TAPS = 9
EPS = 1e-5

# asymmetric conv chunks: top = padded rows 1-15 (15 interior rows),
# bottom = rows 16-28 (13 rows); streams include the 2 pad columns.
CHA, CHB = 15 * W, 13 * W            # 420 / 364 interior px
PCHA, PCHB = 15 * WP, 13 * WP        # 450 / 390 streamed positions
NCHUNK = NLOC * 2                    # 16 chunks per layer

# layer-2 stats engine split: the FIRST ACT_CMG chunk-mg units (which
# complete earliest) use ACT Square+accum (Sy, Syy slots); the rest --
# including the late, collective-critical chunks -- use DVE bn_stats.
ACT_CMG = 8
# binarize layer-2 input on Pool (tensor_tensor is_ge vs broadcast tau)
POOL_BIN = False

F32 = mybir.dt.float32
I16 = mybir.dt.int16
FP16 = mybir.dt.float16
FP8 = mybir.dt.float8e4
AF = mybir.ActivationFunctionType
OP = mybir.AluOpType

# walrus in this container accepts at most ONE sem-wait per instruction;
# hoist extra waits onto same-engine NOPs placed just before.
MAX_WAITS = 1
_split_ctr = [0]


def legalize_waits(nc):
    for fn in nc.m.functions:
        for bb in fn.blocks:
            out = []
            for ins in list(bb.instructions):
                si = ins.sync_info
                if si is not None and len(si.on_wait) > MAX_WAITS:
                    waits = list(si.on_wait)
                    extra, keep = waits[:-MAX_WAITS], waits[-MAX_WAITS:]
                    for w in extra:
                        _split_ctr[0] += 1
                        nop = mybir.InstNoOp(
                            name=f"I-waitsplit-{_split_ctr[0]}", engine=ins.engine
                        )
                        nop.sync_info = mybir.SyncInfo(on_wait=[w], on_update=[])
                        out.append(nop)
                    ins.sync_info = mybir.SyncInfo(
                        on_wait=keep, on_update=list(si.on_update)
                    )
                out.append(ins)
            bb.instructions = out


def build(reps=1):
    nc = bass.Bass()

    xs1_ext = nc.dram_tensor("xs1", [NLOC, P, KG, IMGC], FP8, kind="ExternalInput")
    x_ext = nc.dram_tensor("x", [NLOC, C, H, W], F32, kind="ExternalInput")
    w_ext = {
        l: nc.dram_tensor(f"w{l}b", [KG, P, TAPS, MG * P], FP8, kind="ExternalInput")
        for l in (1, 2)
    }
    gm_ext = {
        l: nc.dram_tensor(f"gamma{l}", [C], F32, kind="ExternalInput") for l in (1, 2)
    }
    bt_ext = {
        l: nc.dram_tensor(f"beta{l}", [C], F32, kind="ExternalInput") for l in (1, 2)
    }
    out_ext = nc.dram_tensor("out", [NLOC, C, H, W], FP16, kind="ExternalOutput")
    cc_in = {l: nc.dram_tensor(f"cc{l}_in", [MG, P, 2], F32) for l in (1, 2)}
    cc_out = {
        l: nc.dram_tensor(f"cc{l}_out", [NCORES, MG, P, 2], F32, addr_space="Shared")
        for l in (1, 2)
    }

    xv = x_ext.rearrange("n c h w -> c n (h w)")    # [256, 8, 784]
    ov = out_ext.rearrange("n c h w -> c n h w")    # [256, 8, 28, 28] fp16

    with TileContext(nc) as tc:
        ctx = ExitStack()
        singles = ctx.enter_context(tc.tile_pool(name="singles", bufs=1))
        aring = ctx.enter_context(tc.tile_pool(name="aring", bufs=2))
        pring = ctx.enter_context(tc.tile_pool(name="pring", bufs=3))
        qring = ctx.enter_context(tc.tile_pool(name="qring", bufs=3))
        outst = ctx.enter_context(tc.tile_pool(name="outst", bufs=3))
        sqscr = ctx.enter_context(tc.tile_pool(name="sqscr", bufs=2))
        small = ctx.enter_context(tc.tile_pool(name="small", bufs=2))
        psum = ctx.enter_context(tc.tile_pool(name="psum", bufs=8, space="PSUM"))

        # ---- persistent tiles -------------------------------------------
        xs = {l: [singles.tile([P, KG, IMGC], FP8, tag=f"xs{l}n{n}", name=f"xs{l}n{n}")
                  for n in range(NLOC)] for l in (1, 2)}
        y = {l: singles.tile([P, MG, NPIX], I16, tag=f"y{l}", name=f"y{l}") for l in (1, 2)}
        wres = singles.tile([P, MG, NPIX], F32, tag="wres", name="wres")
        o1 = singles.tile([P, MG, NPIX], FP16, tag="o1", name="o1")
        wsb = {l: singles.tile([P, TAPS, KG, MG * P], FP8, tag=f"wsb{l}", name=f"wsb{l}") for l in (1, 2)}
        st = {l: singles.tile([P, MG, NCHUNK, 6], F32, tag=f"st{l}", name=f"st{l}") for l in (1, 2)}
        sySlot = singles.tile([P, MG, NCHUNK, 2], F32, tag="sySlot", name="sySlot")
        gmb = {l: singles.tile([P, MG], F32, tag=f"gmb{l}", name=f"gmb{l}") for l in (1, 2)}
        btb = {l: singles.tile([P, MG], F32, tag=f"btb{l}", name=f"btb{l}") for l in (1, 2)}
        epsb = singles.tile([P, 1], F32)
        taub = (singles.tile([P, MG, IMG], F32, tag="taub", name="taub")
                if POOL_BIN else None)

        nc.vector.memset(epsb, EPS)

        # xs2 cells: pads/margins hold 0.5 ({0,1} encoding of sign 0)
        for n in range(NLOC):
            t_ = xs[2][n]
            eng = nc.vector if n % 2 == 0 else nc.gpsimd
            eng.memset(t_[:, :, 0:IOFF + WP], 0.5)          # margin + pad row 0
            eng.memset(t_[:, :, IMGC - 44 - WP:IMGC], 0.5)  # pad row 29 + margin
            for kg in range(KG):
                border = bass.AP(
                    tensor=t_.tensor, offset=t_.offset + kg * IMGC + IOFF + WP,
                    ap=[list(t_.ap[0]), [WP, H], [WP - 1, 2]],
                )
                eng.memset(border, 0.5)

        # ---- weights / host-signed x in (conv1-critical DMAs first) -----
        nc.sync.dma_start(out=xs[1][0], in_=xs1_ext[0])
        for t in range(TAPS):
            for kg in range(KG):
                nc.sync.dma_start(out=wsb[1][:, t, kg, :], in_=w_ext[1][kg][:, t, :])
        for n in range(1, NLOC):
            nc.sync.dma_start(out=xs[1][n], in_=xs1_ext[n])
        for kg in range(KG):
            nc.sync.dma_start(out=wsb[2][:, :, kg, :], in_=w_ext[2][kg])
        for l in (1, 2):
            nc.sync.dma_start(out=gmb[l], in_=gm_ext[l].rearrange("(g p) -> p g", p=P))
            nc.sync.dma_start(out=btb[l], in_=bt_ext[l].rearrange("(g p) -> p g", p=P))
        # x f32 prefetched into wres during phase 1 (overwritten in place
        # by w = s1*y1 + x in phase 2)
        for mg in range(MG):
            nc.sync.dma_start(
                out=wres[:, mg, :].rearrange("p (n q) -> p n q", n=NLOC),
                in_=xv[mg * P:(mg + 1) * P, :, :])

        for _rep in range(reps):
            run_iteration(nc, tc, locals())
        ctx.close()

    legalize_waits(nc)
    return nc


def run_iteration(nc, tc, env):
    g = type("G", (), env)  # attribute access to captured tiles/views

    xs, y, wres, o1, wsb, st = g.xs, g.y, g.wres, g.o1, g.wsb, g.st
    sySlot, gmb, btb, epsb = g.sySlot, g.gmb, g.btb, g.epsb
    xv, ov, cc_in, cc_out = g.xv, g.ov, g.cc_in, g.cc_out
    psum, small = g.psum, g.small
    aring, pring, qring, outst, sqscr = (
        g.aring, g.pring, g.qring, g.outst, g.sqscr)

    # ---- conv chunk: 9 taps x 2 kg-pair DoubleRow matmuls per mg --------
    def conv_chunk(l, n, hb):
        pch = PCHA if hb == 0 else PCHB
        ps = {mg: psum.tile([P, PCHA], F32, tag="ps", name="ps") for mg in range(MG)}
        for t in range(TAPS):
            dy, dx = t // 3 - 1, t % 3 - 1
            q0 = IOFF + WP * (1 + 15 * hb) + WP * dy + dx
            rhs = xs[l][n][:, :, q0:q0 + pch]
            for mg in range(MG):
                lhsT = wsb[l][:, t, :, mg * P:(mg + 1) * P]
                nc.tensor.matmul(
                    ps[mg][:, :pch], lhsT, rhs,
                    start=(t == 0), stop=(t == TAPS - 1),
                    perf_mode=mybir.MatmulPerfMode.DoubleRow,
                )
        return ps

    # copy PSUM->SBUF int16 (layer2: scale=2 -> integer {0,1}-conv fix)
    # + per-chunk stats (DVE bn_stats or ACT Square/accum + copy-accum).
    def chunk_post(l, n, hb, ps):
        ci = 2 * n + hb
        rows = 15 if hb == 0 else 13
        yoff = n * IMG + (CHA if hb == 1 else 0)
        npx = CHA if hb == 0 else CHB
        for mg in range(MG):
            psv = ps[mg][:, : (PCHA if hb == 0 else PCHB)].rearrange(
                "p (r c) -> p r c", c=WP)
            interior = psv[:, :, 1:1 + W]
            ysl = y[l][:, mg, yoff:yoff + npx]
            on_dve = l == 1 or (2 * ci + mg) >= ACT_CMG
            if on_dve:
                nc.scalar.activation(
                    out=ysl.rearrange("p (r c) -> p r c", c=W),
                    in_=interior, func=AF.Copy,
                    scale=1.0 if l == 1 else 2.0,
                )
                nc.vector.bn_stats(out=st[l][:, mg, ci, :], in_=ysl)
            else:
                nc.scalar.activation(
                    out=ysl.rearrange("p (r c) -> p r c", c=W),
                    in_=interior, func=AF.Copy, scale=2.0,
                    accum_out=sySlot[:, mg, ci, 0:1],
                )
                sq = sqscr.tile([P, CHA], F32, tag="sq")
                nc.scalar.activation(
                    out=sq[:, :npx], in_=ysl, func=AF.Square,
                    accum_out=sySlot[:, mg, ci, 1:2],
                )

    # ---- global stats -> affine params ------------------------------
    def stats_and_affine(l):
        # ccsb: per-core contribution (mean, E[y^2]) / NCORES
        ccsb = small.tile([P, MG, 2], F32, tag="ccsb", name="ccsb")
        if l == 1:
            mv = small.tile([P, MG, 2], F32, tag="mv", name="mv")
            for mg in range(MG):
                nc.vector.bn_aggr(out=mv[:, mg, :], in_=st[l][:, mg, :, :])
            msq = small.tile([P, MG, 1], F32, tag="msq", name="msq")
            nc.vector.tensor_tensor(out=msq, in0=mv[:, :, 0:1], in1=mv[:, :, 0:1], op=OP.mult)
            nc.vector.tensor_tensor(out=msq, in0=mv[:, :, 1:2], in1=msq, op=OP.add)
            nc.vector.tensor_scalar(out=ccsb[:, :, 0:1], in0=mv[:, :, 0:1],
                                    scalar1=1.0 / NCORES, scalar2=None, op0=OP.mult)
            nc.vector.tensor_scalar(out=ccsb[:, :, 1:2], in0=msq,
                                    scalar1=1.0 / NCORES, scalar2=None, op0=OP.mult)
        else:
            # group A: chunk-mg < DVE_CMG via bn_aggr; group B: slot sums
            mvA = small.tile([P, MG, 2], F32, tag="mvA", name="mvA")
            cntA = []
            for mg in range(MG):
                cis = [ci for ci in range(NCHUNK) if (2 * ci + mg) >= ACT_CMG]
                cnt = sum(CHA if ci % 2 == 0 else CHB for ci in cis)
                cntA.append(cnt)
                stv = st[l][:, mg, cis[0]:cis[-1] + 1, :]
                nc.vector.bn_aggr(out=mvA[:, mg, :], in_=stv)
            syB = small.tile([P, MG, 2], F32, tag="syB", name="syB")
            for mg in range(MG):
                cis = [ci for ci in range(NCHUNK) if (2 * ci + mg) < ACT_CMG]
                slv = sySlot[:, mg, cis[0]:cis[-1] + 1, :]
                nc.vector.reduce_sum(out=syB[:, mg, :], in_=slv.rearrange("p c d -> p d c"),
                                     axis=mybir.AxisListType.X)
            # totals: S = meanA*cntA + SyB ; Q = (varA+meanA^2)*cntA + SyyB
            mA2 = small.tile([P, MG, 1], F32, tag="mA2", name="mA2")
            nc.vector.tensor_tensor(out=mA2, in0=mvA[:, :, 0:1], in1=mvA[:, :, 0:1], op=OP.mult)
            e2A = small.tile([P, MG, 1], F32, tag="e2A", name="e2A")
            nc.vector.tensor_tensor(out=e2A, in0=mvA[:, :, 1:2], in1=mA2, op=OP.add)
            for mg in range(MG):
                cA = float(cntA[mg])
                sc = 1.0 / (IMG * NLOC * NCORES)
                # ccsb0 = (meanA*cA + SyB) * sc ; ccsb1 = (e2A*cA + SyyB) * sc
                nc.vector.scalar_tensor_tensor(
                    out=ccsb[:, mg, 0:1], in0=mvA[:, mg, 0:1], scalar=cA,
                    in1=syB[:, mg, 0:1], op0=OP.mult, op1=OP.add)
                nc.vector.scalar_tensor_tensor(
                    out=ccsb[:, mg, 1:2], in0=e2A[:, mg, 0:1], scalar=cA,
                    in1=syB[:, mg, 1:2], op0=OP.mult, op1=OP.add)
            nc.vector.tensor_scalar(out=ccsb[:, :, 0:2], in0=ccsb[:, :, 0:2],
                                    scalar1=1.0 / (NPIX * NCORES), scalar2=None, op0=OP.mult)

        nc.sync.dma_start(out=cc_in[l].rearrange("g p d -> p g d"), in_=ccsb)
        nc.gpsimd.collective_compute(
            "AllGather", OP.bypass,
            ins=[cc_in[l][:, :, :]], outs=[cc_out[l][:, :, :, :]],
            replica_groups=[list(range(NCORES))],
        )
        return ccsb

    def affine(l, post_ops=None):
        # per-mg chains so mg0's (s, t) complete while mg1's glr DMA /
        # math are still in flight -- lets the tail start earlier.
        glr = small.tile([P, MG, NCORES, 2], F32, tag="glr", name="glr")
        gl = small.tile([P, MG, 2], F32, tag="gl", name="gl")
        sT = small.tile([P, MG, 1], F32, tag=f"sT{l}", name=f"sT{l}")
        tT = small.tile([P, MG, 1], F32, tag=f"tT{l}", name=f"tT{l}")
        tau = small.tile([P, MG, 1], F32, tag=f"tau{l}", name=f"tau{l}")
        nvar = small.tile([P, MG, 1], F32, tag="nvar", name="nvar")
        sd = small.tile([P, MG, 1], F32, tag="sd", name="sd")
        rinv = small.tile([P, MG, 1], F32, tag="rinv", name="rinv")
        btv = btb[l].rearrange("p (g o) -> p g o", o=1)
        gmv = gmb[l].rearrange("p (g o) -> p g o", o=1)
        for mg in range(MG):
            nc.sync.dma_start(out=glr[:, mg, :, :],
                              in_=cc_out[l][:, mg, :, :].rearrange("r p d -> p r d"))
        for mg in range(MG):
            nc.vector.reduce_sum(out=gl[:, mg, :],
                                 in_=glr[:, mg, :, :].rearrange("p r d -> p d r"),
                                 axis=mybir.AxisListType.X)
            mean, e2 = gl[:, mg, 0:1], gl[:, mg, 1:2]
            nc.vector.scalar_tensor_tensor(
                out=nvar[:, mg, :], in0=mean, scalar=mean,
                in1=e2, op0=OP.mult, op1=OP.subtract)
            nc.scalar.activation(out=sd[:, mg, :], in_=nvar[:, mg, :], func=AF.Sqrt,
                                 bias=epsb, scale=-1.0)
            nc.vector.reciprocal(out=rinv[:, mg, :], in_=sd[:, mg, :])
            nc.vector.tensor_tensor(out=sT[:, mg, :], in0=rinv[:, mg, :],
                                    in1=gmv[:, mg, :], op=OP.mult)
            # tau = s*mu - beta (one fused op); t = -tau
            nc.vector.scalar_tensor_tensor(
                out=tau[:, mg, :], in0=mean, scalar=sT[:, mg, :],
                in1=btv[:, mg, :], op0=OP.mult, op1=OP.subtract)
            nc.vector.tensor_scalar(out=tT[:, mg, :], in0=tau[:, mg, :],
                                    scalar1=-1.0, scalar2=None, op0=OP.mult)
        extra = {} if post_ops != "l1" else {"tau": tau}
        return {"s": sT, "t": tT, **extra}

    # ================== pipeline ==================
    # phase 1: conv1 + copies + stats
    for n in range(NLOC):
        for hb in (0, 1):
            ps = conv_chunk(1, n, hb)
            chunk_post(1, n, hb, ps)

    ccsb1 = stats_and_affine(1)
    aff1 = affine(1, post_ops="l1")
    s1, t1, tau1 = aff1["s"], aff1["t"], aff1["tau"]

    yv = {l: y[l].rearrange("p m (n q) -> p m n q", n=NLOC) for l in (1, 2)}
    wv = wres.rearrange("p m (n q) -> p m n q", n=NLOC)
    o1v = o1.rearrange("p m (n q) -> p m n q", n=NLOC)

    # phase 2: per image: x in, w = x + s1*y1 (f32), binarize {0,1};
    # conv2 chunks trail 2 images behind (software pipeline).
    def b1_image(n):
        xs2v = xs[2][n][:, :, IOFF:IOFF + IMGP].rearrange("p g (r c) -> p g r c", r=HP)
        splits = [(0, 16), (16, H)] if n == 0 else [(0, H)]
        for mg in range(MG):
            wsl = wv[:, mg, n, :]
            for r0, r1 in splits:
                nc.vector.scalar_tensor_tensor(
                    out=wsl[:, r0 * W:r1 * W], in0=yv[1][:, mg, n, r0 * W:r1 * W],
                    scalar=s1[:, mg, :],
                    in1=wsl[:, r0 * W:r1 * W], op0=OP.mult, op1=OP.add)
            for r0, r1 in splits:
                if POOL_BIN and n > 0:
                    nc.gpsimd.tensor_tensor(
                        out=xs2v[:, mg, 1 + r0:1 + r1, 1:1 + W],
                        in0=wsl.rearrange("p (r c) -> p r c", c=W)[:, r0:r1, :],
                        in1=g.taub[:, mg, :].rearrange("p (r c) -> p r c", c=W)[:, r0:r1, :],
                        op=OP.is_ge)
                else:
                    nc.vector.tensor_scalar(
                        out=xs2v[:, mg, 1 + r0:1 + r1, 1:1 + W],
                        in0=wsl.rearrange("p (r c) -> p r c", c=W)[:, r0:r1, :],
                        scalar1=tau1[:, mg, :], scalar2=None, op0=OP.is_ge)

    def conv2_image(n):
        for hb in (0, 1):
            ps = conv_chunk(2, n, hb)
            chunk_post(2, n, hb, ps)

    for n in range(NLOC):
        b1_image(n)
        if POOL_BIN and n == 0:
            for mg in range(MG):
                nc.vector.tensor_scalar(out=g.taub[:, mg, :],
                                        in0=y[1][:, mg, 0:IMG],
                                        scalar1=0.0, scalar2=tau1[:, mg, :],
                                        op0=OP.mult, op1=OP.add)
        if n >= 2:
            conv2_image(n - 2)
    for n in range(NLOC - 2, NLOC):
        conv2_image(n)

    # gap 2: exchange stats; meanwhile o1 = clip(w + t1) (ACT/DVE split)
    ccsb2 = stats_and_affine(2)
    for n2 in range(NLOC // 2):
        lo, hi = 2 * n2 * IMG, (2 * n2 + 2) * IMG
        for mg in range(MG):
            a = aring.tile([P, 2 * IMG], FP16, tag="a")
            nc.scalar.activation(out=a, in_=wres[:, mg, lo:hi], func=AF.Identity,
                                 bias=t1[:, mg, :], scale=1.0)
            nc.vector.tensor_scalar(out=o1[:, mg, lo:hi], in0=a,
                                    scalar1=1.0, scalar2=-1.0, op0=OP.min, op1=OP.max)
    aff2 = affine(2)
    s2, t2 = aff2["s"], aff2["t"]

    # tail: out = clip(s2*y2 + t2 + o1) -> fp16 -> DRAM
    for n2 in range(NLOC // 2):
        lo, hi = 2 * n2 * IMG, (2 * n2 + 2) * IMG
        for mg in range(MG):
            p_ = pring.tile([P, 2 * IMG], FP16, tag="p")
            nc.scalar.activation(out=p_, in_=y[2][:, mg, lo:hi], func=AF.Identity,
                                 bias=t2[:, mg, :], scale=s2[:, mg, :])
            q_ = qring.tile([P, 2 * IMG], FP16, tag="q")
            nc.vector.tensor_tensor(out=q_, in0=p_, in1=o1[:, mg, lo:hi], op=OP.add)
            oc = outst.tile([P, 2 * IMG], FP16, tag="oc")
            nc.vector.tensor_scalar(out=oc, in0=q_, scalar1=1.0, scalar2=-1.0,
                                    op0=OP.min, op1=OP.max)
            for j in (0, 1):
                nc.sync.dma_start(
                    out=ov[mg * P:(mg + 1) * P, 2 * n2 + j, :, :],
                    in_=oc[:, j * IMG:(j + 1) * IMG].rearrange("p (r c) -> p r c", c=W),
                )


_CACHE = {}


def prep_inputs(x, w1, gamma1, beta1, w2, gamma2, beta2):
    fp8np = mybir.dt.np(FP8)

    def prep_w(w):
        wb = np.where(np.asarray(w) >= 0, 1.0, -1.0).astype(np.float32)
        t = wb.reshape(MG, P, KG, P, 3, 3)       # [mg, m, kg, k, ky, kx]
        arr = t.transpose(2, 3, 4, 5, 0, 1)      # [kg, k, ky, kx, mg, m]
        return np.ascontiguousarray(arr.reshape(KG, P, TAPS, MG * P)).astype(fp8np)

    x = np.asarray(x, dtype=np.float32)
    w1b, w2b = prep_w(w1), prep_w(w2)
    g1 = np.asarray(gamma1, np.float32); b1 = np.asarray(beta1, np.float32)
    g2 = np.asarray(gamma2, np.float32); b2 = np.asarray(beta2, np.float32)

    # host-side sign(x) packed into the padded per-image fp8 cell layout
    xs_sign = np.where(x >= 0, 1.0, -1.0).astype(np.float32)
    in_maps = []
    for c in range(NCORES):
        xl = x[c * NLOC:(c + 1) * NLOC]
        sl = xs_sign[c * NLOC:(c + 1) * NLOC]       # [NLOC, C, H, W]
        cell = np.zeros((NLOC, P, KG, IMGC), np.float32)
        s4 = sl.reshape(NLOC, KG, P, H, W)
        pad = np.zeros((NLOC, KG, P, HP, WP), np.float32)
        pad[:, :, :, 1:1 + H, 1:1 + W] = s4
        cell[:, :, :, IOFF:IOFF + IMGP] = (
            pad.transpose(0, 2, 1, 3, 4).reshape(NLOC, P, KG, IMGP))
        in_maps.append({
            "xs1": cell.astype(fp8np),
            "x": np.ascontiguousarray(xl),
            "w1b": w1b, "w2b": w2b,
            "gamma1": g1, "beta1": b1, "gamma2": g2, "beta2": b2,
        })
    return in_maps


def kernel(x, w1, gamma1, beta1, w2, gamma2, beta2):
    if "nc" not in _CACHE:
        _CACHE["nc"] = build()
    nc = _CACHE["nc"]
    in_maps = prep_inputs(x, w1, gamma1, beta1, w2, gamma2, beta2)
    res = run_bass_kernel_spmd(nc, in_maps, core_ids=list(range(NCORES)))
    return np.concatenate(
        [res.results[c]["out"] for c in range(NCORES)], axis=0
    ).astype(np.float32)
